# revision 1
# baseline (speedup 1.0000x reference)
"""Causal single-head attention (B=4, T=4096, C=1024, H=64) on 8 TRN2 NeuronCores.

Sharding: core = 2*b + h handles batch b, t-half h (rows [h*2048, (h+1)*2048)).
Uniform SPMD program per core:
  - triangle: causal attention within the own t-half (s, t both in own half)
  - rect: S^T[s in [0,2048), t in [2048+off, 2048+off+1024)], off = (pid%2)*1024
    (the lower-half keys attending into the upper-half queries, split by t)
Pair AllGathers exchange qT, kT, v; rect partials (num|den) go through a 4th
AllGather and are added (gated by a per-core flag) before the final divide.
Softmax uses no max-subtraction (logits are O(6)); denominator comes from an
appended ones-column in v during the AV matmul.
"""
import sys

sys.path.insert(0, "/opt/trn_rl_repo")

from contextlib import ExitStack

import numpy as np

import concourse.bass as bass
import concourse.mybir as mybir
import concourse.tile as tile
from concourse import bacc
from concourse.bass_utils import run_bass_kernel_spmd

B, T, C, H = 4, 4096, 1024, 64
P = 128
HALF = T // 2              # 2048 rows per core
NB_C = C // P              # 8 contraction tiles
NT = HALF // P             # 16 own t/s tiles
RW = 1024                  # rect t-width per core
NRT = RW // P              # 8 rect t-tiles
SCALE = float(H) ** -0.5
NEG = -1e9
F32, F32R, BF16 = mybir.dt.float32, mybir.dt.float32r, mybir.dt.bfloat16
N_CORES = 8
PAIRS = [[2 * b, 2 * b + 1] for b in range(B)]

# triangle attT storage: s-tile i holds local t-cols [base_i, 2048)
TRI_BASE = [(i // 4) * 512 for i in range(NT)]
TRI_W = [HALF - b for b in TRI_BASE]
TRI_OFF = np.concatenate([[0], np.cumsum(TRI_W)]).tolist()
TRI_TOTAL = TRI_OFF[-1]  # 20480

_CACHE = {}
BODY_REPEAT = 1            # for differential timing in bench.py
PHASES = set(range(1, 9))  # ablation for phase timing
SCHEDULE = None            # list of phase-sets, one body emission each


def build():
    nc = bacc.Bacc("TRN2", target_bir_lowering=False, debug=False,
                   num_devices=N_CORES)
    x = nc.dram_tensor("x", [HALF, C], F32, kind="ExternalInput").ap()
    wq = nc.dram_tensor("wq", [C, H], F32, kind="ExternalInput").ap()
    wk = nc.dram_tensor("wk", [C, H], F32, kind="ExternalInput").ap()
    wv = nc.dram_tensor("wv", [C, H], F32, kind="ExternalInput").ap()
    flag = nc.dram_tensor("flag", [P, 1], F32, kind="ExternalInput").ap()
    trimask = nc.dram_tensor("trimask", [P, P], F32, kind="ExternalInput").ap()
    out = nc.dram_tensor("out", [HALF, H], F32, kind="ExternalOutput").ap()

    with tile.TileContext(nc) as tc, ExitStack() as ctx:
        sb = ctx.enter_context(tc.tile_pool(name="sb", bufs=2))
        stage = ctx.enter_context(tc.tile_pool(name="stage", bufs=4))
        big = ctx.enter_context(tc.tile_pool(name="big", bufs=1))
        ps = ctx.enter_context(tc.tile_pool(name="ps", bufs=2, space="PSUM"))
        dram = ctx.enter_context(tc.tile_pool(name="dram", bufs=1, space="DRAM"))

        # ---- constants ----
        tri_sb = big.tile([P, P], F32, tag="tri")
        nc.sync.dma_start(tri_sb[:], trimask[:])
        flag_sb = big.tile([P, 1], F32, tag="flag")
        nc.sync.dma_start(flag_sb[:], flag[:])
        wqk_sb = big.tile([P, NB_C, 2 * H], BF16, tag="wqk")
        nc.gpsimd.dma_start(wqk_sb[:, :, 0:H], wq.rearrange("(cb p) h -> p cb h", p=P))
        nc.gpsimd.dma_start(wqk_sb[:, :, H:2 * H], wk.rearrange("(cb p) h -> p cb h", p=P))
        wv_sb = big.tile([P, NB_C, H], BF16, tag="wv")
        nc.gpsimd.dma_start(wv_sb[:], wv.rearrange("(cb p) h -> p cb h", p=P))

        schedule = SCHEDULE if SCHEDULE is not None else [PHASES] * BODY_REPEAT
        for _rep in range(len(schedule)):
            cur = schedule[_rep]
            if 1 in cur:
                # ---- x transpose path: per-column-slab cast to bf16 in DRAM
                # (contiguous slab layout), then contiguous DMA-transpose ----
                xbf = dram.tile([NB_C, HALF, P], BF16)
                xT = big.tile([P, NB_C, HALF], BF16, tag="xT")
                for cb in range(NB_C):
                    nc.gpsimd.dma_start(xbf[cb], x[:, cb * P:(cb + 1) * P])
                    nc.sync.dma_start(xT[:, cb, :], xbf[cb], transpose=True)

            if 2 in cur:
                # ---- qk projection: qkT[0:64]=qT, [64:128]=kT (fp32r) ----
                qkT = big.tile([P, HALF], F32R, tag="qkT")
                for tg in range(4):
                    pqk = ps.tile([P, 512], F32, tag="ps")
                    for cb in range(NB_C):
                        nc.tensor.matmul(pqk[:], wqk_sb[:, cb, :],
                                         xT[:, cb, tg * 512:(tg + 1) * 512],
                                         start=(cb == 0), stop=(cb == NB_C - 1))
                    nc.vector.tensor_copy(qkT[:, tg * 512:(tg + 1) * 512], pqk[:])

                # ---- v projection (v_sb[:, st, 0:64]=v, col 64 = ones) ----
                v_sb = big.tile([P, NT, H + 2], BF16, tag="v")
                nc.vector.memset(v_sb[:, :, H:H + 1], 1.0)
                for st in range(NT):
                    pv = ps.tile([P, H], F32, tag="ps")
                    for cb in range(NB_C):
                        nc.tensor.matmul(pv[:], xT[:, cb, st * P:(st + 1) * P],
                                         wv_sb[:, cb, :],
                                         start=(cb == 0), stop=(cb == NB_C - 1))
                    nc.vector.tensor_copy(v_sb[:, st, 0:H], pv[:])

                # kT relocated to partitions 0:64 (matmul needs same base)
                kT = big.tile([H, HALF], F32R, tag="kT")
                nc.sync.dma_start(kT[:], qkT[H:P, :])

            if 3 in cur:
                # ---- pair collectives: gather qT, kT, v ----
                qb = dram.tile([H, HALF], F32R)
                kb = dram.tile([H, HALF], F32R)
                vb = dram.tile([HALF, H], BF16)
                nc.sync.dma_start(qb[:], qkT[0:H, :])
                nc.sync.dma_start(kb[:], qkT[H:P, :])
                nc.sync.dma_start(vb.rearrange("(st p) h -> p st h", p=P),
                                  v_sb[:, :, 0:H])
                gq = dram.tile([2 * H, HALF], F32R)
                gk = dram.tile([2 * H, HALF], F32R)
                gv = dram.tile([T, H], BF16)
                for src, dst in ((qb, gq), (kb, gk), (vb, gv)):
                    nc.gpsimd.collective_compute(
                        "AllGather", mybir.AluOpType.bypass, replica_groups=PAIRS,
                        ins=[src.opt()], outs=[dst.opt()])

                # rect operands: KR/VR = lower-half kT/v (rank0), QR = rank1 qT
                # cols [off, off+RW), off = (pid%2)*RW (dynamic)
                KR = big.tile([H, HALF], F32R, tag="KR")
                nc.sync.dma_start(KR[:], gk[0:H, :])
                VR = big.tile([P, NT, H + 2], BF16, tag="VR")
                nc.vector.memset(VR[:, :, H:H + 1], 1.0)
                nc.sync.dma_start(VR[:, :, 0:H],
                                  gv[0:HALF, :].rearrange("(st p) h -> p st h", p=P))
                QR = big.tile([H, RW], F32R, tag="QR")
                pid = nc.partition_id(engines=[mybir.EngineType.Pool])
                qoff = (pid % 2) * RW
                nc.gpsimd.dma_start(QR[:], gq[H:2 * H, bass.ds(qoff, RW)])

            if 5 in cur:
                # ---- rect QK^T + exp (no mask: s < t always) ----
                attT_rect = big.tile([P, NT * RW], BF16, tag="att_rect")
                for i in range(NT):
                    psr = ps.tile([P, RW], F32, tag="ps")
                    for g in range(RW // 512):
                        nc.tensor.matmul(psr[:, g * 512:(g + 1) * 512],
                                         KR[:, i * P:(i + 1) * P],
                                         QR[:, g * 512:(g + 1) * 512],
                                         start=True, stop=True)
                    nc.scalar.activation(attT_rect[:, i * RW:(i + 1) * RW], psr[:],
                                         mybir.ActivationFunctionType.Exp, scale=SCALE)

            if 7 in cur:
                # ---- rect AV -> partials ----
                rectnd = big.tile([P, NRT, H + 2], F32, tag="rectnd")
                for rt in range(NRT):
                    pr = ps.tile([P, H + 1], F32, tag="ps")
                    for st in range(NT):
                        col = st * RW + rt * P
                        nc.tensor.matmul(pr[:], attT_rect[:, col:col + P],
                                         VR[:, st, 0:H + 1],
                                         start=(st == 0), stop=(st == NT - 1))
                    nc.vector.tensor_copy(rectnd[:, rt, 0:H + 1], pr[:])

            if 8 in cur:
                # ---- partial exchange (flies during the triangle phases) ----
                ndb = dram.tile([RW, H + 2], F32)
                nc.sync.dma_start(ndb.rearrange("(rt p) h -> p rt h", p=P), rectnd[:])
                gnd = dram.tile([HALF, H + 2], F32)
                nc.gpsimd.collective_compute(
                    "AllGather", mybir.AluOpType.bypass, replica_groups=PAIRS,
                    ins=[ndb.opt()], outs=[gnd.opt()])
                gnd_sb = big.tile([P, NT, H + 2], F32, tag="gnd")
                nc.sync.dma_start(gnd_sb[:], gnd.rearrange("(tt p) h -> p tt h", p=P))

            if 4 in cur:
                # ---- triangle QK^T (S^T layout) + exp ----
                attT_tri = big.tile([P, TRI_TOTAL], BF16, tag="att_tri")
                for i in range(NT):
                    base, w = TRI_BASE[i], TRI_W[i]
                    pst = ps.tile([P, w], F32, tag="ps")
                    for g in range(base // 512, 4):
                        nc.tensor.matmul(pst[:, g * 512 - base:(g + 1) * 512 - base],
                                         kT[:, i * P:(i + 1) * P],
                                         qkT[0:H, g * 512:(g + 1) * 512],
                                         start=True, stop=True)
                    d0 = i * P - base
                    nc.vector.tensor_add(pst[:, d0:d0 + P], pst[:, d0:d0 + P], tri_sb[:])
                    nc.scalar.activation(attT_tri[:, TRI_OFF[i]:TRI_OFF[i] + w], pst[:],
                                         mybir.ActivationFunctionType.Exp, scale=SCALE)

            if 6 in cur:
                # ---- triangle AV (num|den via ones column) ----
                trind = big.tile([P, NT, H + 2], F32, tag="trind")
                for tt in range(NT):
                    po = ps.tile([P, H + 1], F32, tag="ps")
                    for st in range(tt + 1):
                        col = TRI_OFF[st] + tt * P - TRI_BASE[st]
                        nc.tensor.matmul(po[:], attT_tri[:, col:col + P],
                                         v_sb[:, st, 0:H + 1],
                                         start=(st == 0), stop=(st == tt))
                    nc.vector.tensor_copy(trind[:, tt, 0:H + 1], po[:])

            if 8 in cur:
                # ---- final: nd = tri + flag*gathered; out = num/den ----
                for tt in range(NT):
                    tmp = sb.tile([P, H + 1], F32, tag="tmp")
                    nc.vector.tensor_scalar_mul(tmp[:], gnd_sb[:, tt, 0:H + 1],
                                                flag_sb[:, 0:1])
                    ndf = sb.tile([P, H + 1], F32, tag="ndf")
                    nc.vector.tensor_add(ndf[:], trind[:, tt, 0:H + 1], tmp[:])
                    rec = sb.tile([P, 1], F32, tag="rec")
                    nc.vector.reciprocal(rec[:], ndf[:, H:H + 1])
                    ot = sb.tile([P, H], F32, tag="ot")
                    nc.vector.tensor_scalar_mul(ot[:], ndf[:, 0:H], rec[:, 0:1])
                    nc.sync.dma_start(out[tt * P:(tt + 1) * P, :], ot[:])

    nc.compile()
    return nc


def kernel(x, Wq, Wk, Wv):
    x = np.asarray(x, dtype=np.float32)
    Wq = np.asarray(Wq, dtype=np.float32)
    Wk = np.asarray(Wk, dtype=np.float32)
    Wv = np.asarray(Wv, dtype=np.float32)
    if "nc" not in _CACHE:
        _CACHE["nc"] = build()
    nc = _CACHE["nc"]

    # S^T layout: partition=s, free=t; allowed s<=t -> tri[s,t]=0 iff s<=t
    tri = np.where(np.arange(P)[:, None] <= np.arange(P)[None, :], 0.0,
                   NEG).astype(np.float32)
    in_maps = []
    for c in range(N_CORES):
        b, h = c // 2, c % 2
        in_maps.append({
            "x": np.ascontiguousarray(x[b, h * HALF:(h + 1) * HALF, :]),
            "wq": Wq, "wk": Wk, "wv": Wv,
            "flag": np.full((P, 1), float(h), np.float32),
            "trimask": tri,
        })
    res = None
    for attempt in range(4):
        try:
            res = run_bass_kernel_spmd(nc, in_maps, list(range(N_CORES)))
            break
        except Exception:
            if attempt == 3:
                raise
            import time as _time
            _time.sleep(5)
    out = np.empty((B, T, H), np.float32)
    for c in range(N_CORES):
        b, h = c // 2, c % 2
        out[b, h * HALF:(h + 1) * HALF, :] = res.results[c]["out"]
    return out



# revision 2
# speedup vs baseline: 1.1336x; 1.1336x over previous
"""Causal single-head attention (B=4, T=4096, C=1024, H=64) on 8 TRN2 cores.

v2: core = 2*b + h handles batch b, t-half h. Per-core program (SPMD):
  - x-prep: 4 row-block cast DMAs fp32->bf16 (DRAM->DRAM), then per
    (slab, row-block) strided DMA-transposes into xT [P, 8, 2048] bf16,
    pipelined with the projections.
  - proj: qkT f32r [P, 2048] (q rows 0:64, k rows 64:128), plus bf16 copies
    QRb [64, 2048] (q) and gated kbf[64:128] (k*(1-flag)) for the rect,
    v_sb with appended ones-columns, gated kvg_v.
  - TWO pair ReduceScatter(add) collectives (s-halves of packed (k|v)):
    lower core contributes its gated (k|v), upper contributes zeros; both
    rank slots carry the same payload so out_rank = kv_lower on both sides.
    Part A (s in [0,1024)) kicks as soon as its kv half is projected, so
    rect can start while part B is still in flight. No q exchange, no
    partial exchange.
  - triangle: causal attention within own half (f32r QK, exact-width trim).
  - rect: FULL 2048-wide rect (own queries x lower-half keys) on every core;
    on lower cores this computes garbage that the flag gates out.
  - final: nd = trind + flag*rectnd; out = num/den (interleaved with AV).
"""
import sys

sys.path.insert(0, "/opt/trn_rl_repo")

from contextlib import ExitStack

import numpy as np

import concourse.bass as bass
import concourse.mybir as mybir
import concourse.tile as tile
from concourse import bacc
from concourse.bass_utils import run_bass_kernel_spmd

B, T, C, H = 4, 4096, 1024, 64
P = 128
HALF = T // 2              # 2048 rows per core
NB_C = C // P              # 8 contraction tiles
NT = HALF // P             # 16 own t/s tiles
SCALE = float(H) ** -0.5
NEG = -1e9
F32, F32R, BF16 = mybir.dt.float32, mybir.dt.float32r, mybir.dt.bfloat16
N_CORES = 8
PAIRS = [[2 * b, 2 * b + 1] for b in range(B)]

# triangle attT storage: s-tile i holds local t-cols [TRI_BASE[i], 2048)
TRI_BASE = [(i // 4) * 512 for i in range(NT)]
TRI_W = [HALF - b for b in TRI_BASE]
TRI_OFF = np.concatenate([[0], np.cumsum(TRI_W)]).tolist()
TRI_TOTAL = TRI_OFF[-1]  # 20480

# packed kv payload per s-half (bf16 elems): k [64,1024] then v [128, 8, 66]
KV_K = 64 * 1024                 # 65536
KV_V = P * (NT // 2) * (H + 2)   # 67584
KV_N = KV_K + KV_V

_CACHE = {}
BODY_REPEAT = 1
PHASES = set(range(1, 9))
SCHEDULE = None


def build():
    nc = bacc.Bacc("TRN2", target_bir_lowering=False, debug=False,
                   num_devices=N_CORES)
    x = nc.dram_tensor("x", [HALF, C], F32, kind="ExternalInput").ap()
    wq = nc.dram_tensor("wq", [C, H], F32, kind="ExternalInput").ap()
    wk = nc.dram_tensor("wk", [C, H], F32, kind="ExternalInput").ap()
    wv = nc.dram_tensor("wv", [C, H], F32, kind="ExternalInput").ap()
    flag = nc.dram_tensor("flag", [P, 1], F32, kind="ExternalInput").ap()
    glow = nc.dram_tensor("glow", [P, 1], F32, kind="ExternalInput").ap()
    trimask = nc.dram_tensor("trimask", [P, P], F32, kind="ExternalInput").ap()
    out = nc.dram_tensor("out", [HALF, H], F32, kind="ExternalOutput").ap()

    with tile.TileContext(nc) as tc, ExitStack() as ctx:
        sb = ctx.enter_context(tc.tile_pool(name="sb", bufs=2))
        big = ctx.enter_context(tc.tile_pool(name="big", bufs=1))
        ps = ctx.enter_context(tc.tile_pool(name="ps", bufs=2, space="PSUM"))
        dram = ctx.enter_context(tc.tile_pool(name="dram", bufs=1, space="DRAM"))

        # ---- constants (on scalar queue: keeps SP/Pool free for x-prep) ----
        tri_sb = big.tile([P, P], F32, tag="tri")
        nc.scalar.dma_start(tri_sb[:], trimask[:])
        flag_sb = big.tile([P, 1], F32, tag="flag")
        nc.scalar.dma_start(flag_sb[:], flag[:])
        glow_sb = big.tile([P, 1], F32, tag="glow")
        nc.scalar.dma_start(glow_sb[:], glow[:])
        wqk_sb = big.tile([P, NB_C, 2 * H], BF16, tag="wqk")
        nc.gpsimd.dma_start(wqk_sb[:, :, 0:H], wq.rearrange("(cb p) h -> p cb h", p=P))
        nc.gpsimd.dma_start(wqk_sb[:, :, H:2 * H], wk.rearrange("(cb p) h -> p cb h", p=P))
        wv_sb = big.tile([P, NB_C, H], BF16, tag="wv")
        nc.gpsimd.dma_start(wv_sb[:], wv.rearrange("(cb p) h -> p cb h", p=P))

        schedule = SCHEDULE if SCHEDULE is not None else [PHASES] * BODY_REPEAT
        for _rep in range(len(schedule)):
            cur = schedule[_rep]
            if 1 in cur or 2 in cur:
                xbf = dram.tile([HALF, C], BF16)
                xT = big.tile([P, NB_C, HALF], BF16, tag="xT")
                qkT = big.tile([P, HALF], F32R, tag="qkT")
                QRb = big.tile([H, HALF], BF16, tag="QRb")
                kbf = big.tile([P, HALF], BF16, tag="kbf")
                v_sb = big.tile([P, NT, H + 2], BF16, tag="v")
                kvg_v = big.tile([P, NT, H + 2], BF16, tag="kvg_v")

            if 2 in cur:
                nc.vector.memset(v_sb[:, :, H:H + 2], 1.0)
                nc.vector.tensor_scalar_mul(kvg_v[:, :, H:H + 2],
                                            v_sb[:, :, H:H + 2], glow_sb[:, 0:1])

            # ---- x-prep + proj, pipelined per 1024-row half; cast-h0,
            # transposes-h0, cast-h1, transposes-h1 all in order on SP so
            # the h1 cast cannot steal the DMA engines from h0 transposes.
            for half in range(2):
                lo = half * 1024
                if 1 in cur:
                    nc.gpsimd.dma_start(xbf[lo:lo + 1024, :], x[lo:lo + 1024, :])
                    for cb in range(NB_C):
                        nc.sync.dma_start(
                            xT[:, cb, lo:lo + 1024],
                            xbf[lo:lo + 1024, cb * P:(cb + 1) * P],
                            transpose=True)
                if 2 in cur:
                    for tg in (2 * half, 2 * half + 1):
                        pqk = ps.tile([P, 512], F32, tag="ps")
                        for cb in range(NB_C):
                            nc.tensor.matmul(pqk[:], wqk_sb[:, cb, :],
                                             xT[:, cb, tg * 512:(tg + 1) * 512],
                                             start=(cb == 0), stop=(cb == NB_C - 1))
                        nc.vector.tensor_copy(qkT[:, tg * 512:(tg + 1) * 512], pqk[:])
                        nc.vector.tensor_copy(QRb[:, tg * 512:(tg + 1) * 512],
                                              pqk[0:H, :])
                        nc.vector.tensor_scalar_mul(kbf[H:P, tg * 512:(tg + 1) * 512],
                                                    pqk[H:P, :], glow_sb[H:P, 0:1])
                    for sg in range(2 * half, 2 * half + 2):
                        pv4 = ps.tile([P, 4, H], F32, tag="psv")
                        for j in range(4):
                            st = 4 * sg + j
                            for cb in range(NB_C):
                                nc.tensor.matmul(pv4[:, j, :],
                                                 xT[:, cb, st * P:(st + 1) * P],
                                                 wv_sb[:, cb, :],
                                                 start=(cb == 0), stop=(cb == NB_C - 1))
                        nc.vector.tensor_copy(
                            v_sb[:, 4 * sg:4 * sg + 4, 0:H], pv4[:])
                        nc.vector.tensor_scalar_mul(
                            kvg_v[:, 4 * sg:4 * sg + 4, 0:H], pv4[:],
                            glow_sb[:, 0:1])

                if 3 in cur:
                    # ---- kv s-half finished -> stage + ReduceScatter ----
                    # all on Pool: its later work (next stage/load) is gated
                    # on the same collectives anyway, so blocking is free.
                    kvd = dram.tile([2, KV_N], BF16)
                    for slot in range(2):
                        nc.gpsimd.dma_start(
                            kvd[slot, 0:KV_K].rearrange("(p t) -> p t", p=H),
                            kbf[H:P, lo:lo + 1024])
                        nc.gpsimd.dma_start(
                            kvd[slot, KV_K:KV_N].rearrange(
                                "(p st h) -> p st h", p=P, st=NT // 2),
                            kvg_v[:, 8 * half:8 * half + 8, :])
                    kvr = dram.tile([KV_N], BF16)
                    nc.gpsimd.collective_compute(
                        "ReduceScatter", mybir.AluOpType.add,
                        replica_groups=PAIRS,
                        ins=[kvd.opt()], outs=[kvr.opt()])
                    KR = big.tile([H, 1024], BF16, tag=f"KR{half}")
                    nc.gpsimd.dma_start(
                        KR[:], kvr[0:KV_K].rearrange("(p t) -> p t", p=H))
                    VR = big.tile([P, NT // 2, H + 2], BF16, tag=f"VR{half}")
                    nc.gpsimd.dma_start(
                        VR[:], kvr[KV_K:KV_N].rearrange(
                            "(p st h) -> p st h", p=P, st=NT // 2))
                    if half == 0:
                        KRA, VRA = KR, VR
                    else:
                        KRB, VRB = KR, VR

            if 2 in cur:
                # kT relocated to partitions 0:64 for the triangle
                kT = big.tile([H, HALF], F32R, tag="kT")
                nc.sync.dma_start(kT[:], qkT[H:P, :])

            if 4 in cur:
                # ---- triangle QK^T (S^T layout) + exp, exact-width trim ----
                attT_tri = big.tile([P, TRI_TOTAL], BF16, tag="att_tri")
                for i in range(NT):
                    base = TRI_BASE[i]
                    t0 = i * P           # first valid t column
                    for g in range(t0 // 512, 4):
                        lo = max(t0, g * 512)
                        hi = (g + 1) * 512
                        pst = ps.tile([P, 512], F32, tag="ps")
                        nc.tensor.matmul(pst[:, 0:hi - lo],
                                         kT[:, i * P:(i + 1) * P],
                                         qkT[0:H, lo:hi],
                                         start=True, stop=True)
                        if lo == t0:  # chunk containing the diagonal block
                            nc.vector.tensor_add(pst[:, 0:P], pst[:, 0:P], tri_sb[:])
                        nc.scalar.activation(
                            attT_tri[:, TRI_OFF[i] + lo - base:TRI_OFF[i] + hi - base],
                            pst[:, 0:hi - lo],
                            mybir.ActivationFunctionType.Exp, scale=SCALE)

            if 5 in cur:
                # ---- rect QK^T + exp, s-half A (starts once RS-A lands) ----
                attT_rect = big.tile([P, NT, HALF], BF16, tag="att_rect")
                for i in range(NT // 2):
                    for g in range(4):
                        psr = ps.tile([P, 512], F32, tag="ps")
                        nc.tensor.matmul(psr[:],
                                         KRA[:, i * P:(i + 1) * P],
                                         QRb[:, g * 512:(g + 1) * 512],
                                         start=True, stop=True)
                        nc.scalar.activation(
                            attT_rect[:, i, g * 512:(g + 1) * 512], psr[:],
                            mybir.ActivationFunctionType.Exp, scale=SCALE)

            if 6 in cur:
                # ---- triangle AV (num|den via ones column) ----
                trind = big.tile([P, NT, H + 2], F32, tag="trind")
                for tt in range(NT):
                    po = ps.tile([P, H + 1], F32, tag="ps")
                    for st in range(tt + 1):
                        col = TRI_OFF[st] + tt * P - TRI_BASE[st]
                        nc.tensor.matmul(po[:], attT_tri[:, col:col + P],
                                         v_sb[:, st, 0:H + 1],
                                         start=(st == 0), stop=(st == tt))
                    nc.vector.tensor_copy(trind[:, tt, 0:H + 1], po[:])

            if 5 in cur:
                # ---- rect QK^T + exp, s-half B ----
                for i in range(NT // 2):
                    for g in range(4):
                        psr = ps.tile([P, 512], F32, tag="ps")
                        nc.tensor.matmul(psr[:],
                                         KRB[:, i * P:(i + 1) * P],
                                         QRb[:, g * 512:(g + 1) * 512],
                                         start=True, stop=True)
                        nc.scalar.activation(
                            attT_rect[:, 8 + i, g * 512:(g + 1) * 512], psr[:],
                            mybir.ActivationFunctionType.Exp, scale=SCALE)

            if 7 in cur:
                # ---- rect AV + final combine, interleaved per t-tile ----
                for tt in range(NT):
                    pr = ps.tile([P, H + 1], F32, tag="ps")
                    for st in range(NT):
                        vv = VRA if st < 8 else VRB
                        nc.tensor.matmul(pr[:],
                                         attT_rect[:, st, tt * P:(tt + 1) * P],
                                         vv[:, st % 8, 0:H + 1],
                                         start=(st == 0), stop=(st == NT - 1))
                    if 8 in cur:
                        tmp = sb.tile([P, H + 1], F32, tag="tmp")
                        nc.vector.tensor_scalar_mul(tmp[:], pr[:], flag_sb[:, 0:1])
                        ndf = sb.tile([P, H + 1], F32, tag="ndf")
                        nc.vector.tensor_add(ndf[:], trind[:, tt, 0:H + 1], tmp[:])
                        rec = sb.tile([P, 1], F32, tag="rec")
                        nc.vector.reciprocal(rec[:], ndf[:, H:H + 1])
                        ot = sb.tile([P, H], F32, tag="ot")
                        nc.vector.tensor_scalar_mul(ot[:], ndf[:, 0:H], rec[:, 0:1])
                        nc.sync.dma_start(out[tt * P:(tt + 1) * P, :], ot[:])

    nc.compile()
    return nc


def make_in_maps(x, Wq, Wk, Wv):
    x = np.asarray(x, dtype=np.float32)
    Wq = np.asarray(Wq, dtype=np.float32)
    Wk = np.asarray(Wk, dtype=np.float32)
    Wv = np.asarray(Wv, dtype=np.float32)
    tri = np.where(np.arange(P)[:, None] <= np.arange(P)[None, :], 0.0,
                   NEG).astype(np.float32)
    in_maps = []
    for c in range(N_CORES):
        b, h = c // 2, c % 2
        in_maps.append({
            "x": np.ascontiguousarray(x[b, h * HALF:(h + 1) * HALF, :]),
            "wq": Wq, "wk": Wk, "wv": Wv,
            "flag": np.full((P, 1), float(h), np.float32),
            "glow": np.full((P, 1), 1.0 - float(h), np.float32),
            "trimask": tri,
        })
    return in_maps


def kernel(x, Wq, Wk, Wv):
    if "nc" not in _CACHE:
        _CACHE["nc"] = build()
    nc = _CACHE["nc"]
    in_maps = make_in_maps(x, Wq, Wk, Wv)
    res = None
    for attempt in range(4):
        try:
            res = run_bass_kernel_spmd(nc, in_maps, list(range(N_CORES)))
            break
        except Exception:
            if attempt == 3:
                raise
            import time as _time
            _time.sleep(5)
    out = np.empty((B, T, H), np.float32)
    for c in range(N_CORES):
        b, h = c // 2, c % 2
        out[b, h * HALF:(h + 1) * HALF, :] = res.results[c]["out"]
    return out
